# revision 11
# baseline (speedup 1.0000x reference)
"""CoxPH loss (nn_CoxPHLoss) on 8 Trainium2 NeuronCores via Bass.

Contract: kernel(risk, time, event) -> np.float32 scalar, matching

    order = argsort(-time); r = risk[order]; e = event[order] > 0
    clse = cumulative logsumexp of r (descending-time order)
    log_denom_i = clse[last index of i's time-tie group]
    nll = sum_{i: e_i} (log_denom_i - r_i)      (0.0 if no events)

Because time takes integer values in [0, 4096), the tie-group denominator
for time value t is SE_t = sum_{j: time_j >= t} exp(risk_j), so

    nll = sum_t d_t * log(SE_t) - sum_i event_i * risk_i,  d_t = #events at t.

Distribution (per the data-parallel sharding hint): the host performs the
descending-time sort as the sharding step (16-bit-key radix argsort) and
splits the sorted stream over the 8 cores. Each core runs the memory-bound
scan pass over its 1M-sample shard:
  - exp(risk) on ScalarE (fp16 in/out),
  - consecutive-pair sums via one unit-stride fp16 add (VectorE 2x mode;
    the host de-interleaves each chunk so pairs split into halves),
  - per-partition-row inclusive prefix sums over the pair sums via VectorE
    tensor_tensor_scan (fp32 carry state, rows chained across chunks) --
    the per-shard scan at half the element count,
  - sum(event*risk) partials: fp16 product on VectorE, reduced by the
    otherwise-idle TensorE via a ones-vector matmul accumulated in PSUM,
  - the prefix array written back downsampled 8x in fp16 (strided downcast
    on ScalarE) plus exact fp32 per-row totals.
The cross-shard "carry exchange" is the O(#rows)=O(1024) float64 exclusive
prefix over per-row totals on the host, which also rebuilds each time
group's boundary prefix from the downsampled value plus <=7 exp() terms,
then takes the final all-reduce sum (an O(4096) dot).
"""

import sys

sys.path.insert(0, "/opt/trn_rl_repo")

import numpy as np

import concourse.bacc as bacc
import concourse.mybir as mybir
import concourse.tile as tile
from concourse import bass_utils

P = 128            # SBUF partitions
N_CORES = 8
T_MAX = 4096
FTOT = 8192        # free elems per partition-row (per core: P*FTOT = 1M)
FC = 2048          # chunk of the free dim per iteration
NCH = FTOT // FC
DS = 8             # prefix writeback downsample factor
N = N_CORES * P * FTOT

_cache = {}


def _build_kernel():
    """Per-core SPMD kernel (flat [P, FTOT] layout, column-slice chunks).

    in:  r [P,FTOT] fp16 (sorted risks), e [P,FTOT] fp16 (sorted events)
    out: t1 [P,FTOT/DS] fp16 -- inclusive prefix sums of exp(r) along each
         partition-row at every DS-th position; rowlast [P,1] f32 -- exact
         row totals; er [1,512] f32 -- PSUM partials of sum(r*e).
    """
    nc = bacc.Bacc("TRN2", target_bir_lowering=False, debug=False)
    r_d = nc.dram_tensor("r", [P, FTOT], mybir.dt.float16, kind="ExternalInput")
    e_d = nc.dram_tensor("e", [P, FTOT], mybir.dt.float16, kind="ExternalInput")
    t1_d = nc.dram_tensor("t1", [P, FTOT // DS], mybir.dt.float16,
                          kind="ExternalOutput")
    er_d = nc.dram_tensor("er", [1, 512], mybir.dt.float32, kind="ExternalOutput")
    rl_d = nc.dram_tensor("rowlast", [P, 1], mybir.dt.float32,
                          kind="ExternalOutput")

    with tile.TileContext(nc) as tc:
        with (
            tc.tile_pool(name="io", bufs=6) as io,
            tc.tile_pool(name="work", bufs=4) as work,
            tc.tile_pool(name="acc", bufs=1) as acc,
            tc.tile_pool(name="psum", bufs=1, space="PSUM") as psum,
        ):
            ones_w = acc.tile([P, 1], mybir.dt.float16)
            nc.gpsimd.memset(ones_w[:], 1.0)
            er_ps = psum.tile([1, 512], mybir.dt.float32)
            prev = None
            for c in range(NCH):
                off = c * FC
                rt = io.tile([P, FC], mybir.dt.float16, tag="rt")
                nc.sync.dma_start(rt[:], r_d[:, off : off + FC])
                et = io.tile([P, FC], mybir.dt.float16, tag="et")
                nc.sync.dma_start(et[:], e_d[:, off : off + FC])

                ex = work.tile([P, FC], mybir.dt.float16, tag="ex")
                nc.scalar.activation(ex[:], rt[:], mybir.ActivationFunctionType.Exp)

                # chunk data is host-de-interleaved: position j < FC/2 holds
                # sorted element 2j, position FC/2+j holds 2j+1 -> a single
                # unit-stride fp16 add (DVE 2x) forms consecutive-pair sums,
                # and the scan then covers half the elements.
                ps = work.tile([P, FC // 2], mybir.dt.float16, tag="ps")
                nc.vector.tensor_tensor(ps[:], ex[:, : FC // 2], ex[:, FC // 2 :],
                                        mybir.AluOpType.add)
                t1 = work.tile([P, FC // 2], mybir.dt.float32, tag="t1")
                init = 0.0 if prev is None else prev
                nc.vector.tensor_tensor_scan(
                    t1[:], ps[:], ps[:], init,
                    mybir.AluOpType.add, mybir.AluOpType.bypass,
                )
                prev = t1[:, FC // 2 - 1 : FC // 2]

                t1h = work.tile([P, FC // DS], mybir.dt.float16, tag="t1h")
                nc.scalar.copy(t1h[:], t1[:, DS // 2 - 1 : FC // 2 : DS // 2])
                nc.sync.dma_start(t1_d[:, off // DS : (off + FC) // DS], t1h[:])

                # er partials: fp16 elementwise product (DVE 2x), then the
                # otherwise-idle TensorE reduces via a ones-matmul into PSUM
                er_s = work.tile([P, FC], mybir.dt.float16, tag="ers")
                nc.vector.tensor_tensor(er_s[:], rt[:], et[:],
                                        mybir.AluOpType.mult)
                for j in range(FC // 512):
                    nc.tensor.matmul(
                        er_ps[:], ones_w[:], er_s[:, j * 512 : (j + 1) * 512],
                        start=(c == 0 and j == 0),
                        stop=(c == NCH - 1 and j == FC // 512 - 1))

            nc.sync.dma_start(rl_d[:], prev)
            er_sb = acc.tile([1, 512], mybir.dt.float32)
            nc.scalar.copy(er_sb[:], er_ps[:])
            nc.sync.dma_start(er_d[:], er_sb[:])

    nc.compile()
    return nc


def _get_kernel():
    if "nc" not in _cache:
        _cache["nc"] = _build_kernel()
    return _cache["nc"]


def _run_device_pass(r16: np.ndarray, e16: np.ndarray):
    """r16/e16: fp16 [N] sorted. Returns (t1ds_flat fp16 [N/DS],
    row_tot f64 [N_CORES*P], er_total float)."""
    per_core = P * FTOT
    nc = _get_kernel()

    in_maps = []
    for c in range(N_CORES):
        sh = slice(c * per_core, (c + 1) * per_core)
        in_maps.append({"r": r16[sh].reshape(P, FTOT),
                        "e": e16[sh].reshape(P, FTOT)})

    res = bass_utils.run_bass_kernel_spmd(
        nc, in_maps, core_ids=list(range(N_CORES)))

    t1_parts, rl_parts = [], []
    er_total = 0.0
    for c in range(N_CORES):
        out = res.results[c]
        t1_parts.append(np.asarray(out["t1"]).reshape(per_core // DS))
        rl_parts.append(np.asarray(out["rowlast"]).reshape(P))
        er_total += float(np.asarray(out["er"]).astype(np.float64).sum())
    return (np.concatenate(t1_parts),
            np.concatenate(rl_parts).astype(np.float64), er_total)


def kernel(risk: np.ndarray, time: np.ndarray, event: np.ndarray) -> np.float32:
    risk = np.asarray(risk, dtype=np.float32)
    time = np.asarray(time)
    event = np.asarray(event)
    if time.dtype.kind == "u":          # unsigned would wrap under negation
        time = time.astype(np.int64)
    assert risk.shape[0] == N, f"expected N={N}, got {risk.shape}"

    if int((event > 0).sum()) == 0:
        return np.float32(0.0)

    # host sharding: descending-time sort (16-bit-key radix argsort)
    order = np.argsort((-time).astype(np.int16), kind="stable")
    r16 = risk[order].astype(np.float16)
    e16 = (event[order] > 0).astype(np.float16)

    # de-interleave each (row, chunk) segment: [e0 e2 e4 ... | e1 e3 e5 ...]
    # so the device pair-sum add is unit-stride (see _build_kernel)
    def _deint(x):
        return np.ascontiguousarray(
            x.reshape(-1, NCH, FC // 2, 2).transpose(0, 1, 3, 2)).reshape(-1)
    r16d = _deint(r16)
    e16d = _deint(e16)

    t1ds_flat, row_tot, er_total = _run_device_pass(r16d, e16d)

    # host combine: O(#rows + T_MAX)
    base = np.concatenate([[0.0], np.cumsum(row_tot)[:-1]])

    cnt_desc = np.bincount(time, minlength=T_MAX)[::-1]     # t = T_MAX-1 first
    ends = np.cumsum(cnt_desc)                              # 1-based group ends
    d_desc = np.bincount(time[event > 0], minlength=T_MAX)[::-1].astype(np.float64)

    mask = d_desc > 0
    s = ends[mask] - 1                                      # last index of group
    row = s // FTOT
    f = s % FTOT
    j = (f + 1) // DS - 1                                   # downsampled index
    ds_val = np.where(
        j >= 0, t1ds_flat[row * (FTOT // DS) + np.maximum(j, 0)],
        np.float16(0.0)).astype(np.float64)
    tail = np.zeros(len(s), dtype=np.float64)               # <= DS-1 exp terms
    start = row * FTOT + (j + 1) * DS
    for k in range(len(s)):
        lo, hi = start[k], s[k] + 1
        if hi > lo:
            tail[k] = np.exp(r16[lo:hi].astype(np.float64)).sum()

    se = base[row] + ds_val + tail
    nll = float(np.dot(d_desc[mask], np.log(se))) - er_total
    return np.float32(nll)
